# revision 40
# baseline (speedup 1.0000x reference)
"""Contrastive loss kernel for Trainium2 (8 NeuronCores).

loss = mean((sim.sum(-1) - diag) / T) with sim = n @ n.T, n = x/||x||
     = (||s||^2 - N) / (N*T)          with s = sum_i x_i / ||x_i||

Each core takes a [2048, 512] row shard shipped as fp16 packed
[128, 8192] (partition p holds rows 16p..16p+15; "segment" t = row t
within each partition).  Columns are reordered on the host so the
stream tail is tiny: late segments are split into an early "head"
(shipped in the first chunk) and a small "tail" (shipped last), so the
only work after the final DMA chunk lands is a short fold + rsqrt +
four tiny matmuls.

Per-segment row sum-of-squares runs on three engines in parallel (DVE:
2x-mode square + 4x-mode reduce; ACT: Square activation with
accum_out; Pool: square only — TensorScalarPtr is not a legal Pool
opcode — with the reduce on DVE), balanced so everything hides under
the serial DMA stream.  Split segments reduce head and tail into
separate accumulators merged by a tiny DVE add before the sqrt batch
(tensor_tensor_reduce would fold them in one op but crashes the
device runtime).

rn = 1/sqrt(ss) via ACT sqrt + DVE reciprocal in batches; a dummy sqrt
on a const AP at the head pins the single activation-table load
(sqrt_and_others covers Sqrt and Square) before data arrives.

The weighted row sum s = sum_t x_t^T @ rn_t runs on the PE as 64 tiny
matmuls (x tile stationary, rn column moving), accumulating into one
[128, 4, 512] PSUM tile (bank c holds dim chunk c).  One DVE op gathers
the four bank columns into SBUF.

The result leaves via a SWDGE kv_writeback whose descriptors are
prepared on Pool during the DMA head and fired by trigger_dma at the
end — skipping the ~1.3us HWDGE+DGE latency a plain output DMA would
pay after the final compute.  The trigger is gated on the res copy by
declaring a write of a gate tile whose last writer follows the copy in
DVE program order: the WAW edge rides Tile's own lane semaphores,
which are reset every launch (a user semaphore's stale value from a
previous execution let the trigger fire early on hardware).  The host
sums the 8 per-core partials and applies the scalar epilogue.
"""

import numpy as np

import concourse.bass as bass
import concourse.bacc as bacc
import concourse.tile as tile
from concourse import mybir
from concourse.bass_utils import run_bass_kernel_spmd

N = 16384
D = 512
NCORES = 8
ROWS = N // NCORES    # 2048 rows per core
P = 128               # SBUF partitions
NSEG = ROWS // P      # 16 segments of [128, 512] per core
DCH = D // P          # 4 psum chunks of 128 dims
FLAT = NSEG * D       # 8192 flat columns per partition
TEMPERATURE = 0.5

F32 = mybir.dt.float32
F16 = mybir.dt.float16
I32 = mybir.dt.int32
SQUARE = mybir.ActivationFunctionType.Square
MULT = mybir.AluOpType.mult
ADD = mybir.AluOpType.add

# --- configuration -------------------------------------------------------
# SPLITS: segments shipped as head (early) + tail (late); 15 must be last.
# TAILS[t]: tail width in columns.  FULL_ENG: engine for each unsplit
# segment ('d'/'a'/'p').  FOLD_ENG: engine folding each split tail
# (seg 15 is always a DVE tensor_tensor_reduce).  TAIL_CHUNKS: how the
# tails are grouped into trailing DMA chunks.  FULL_PER_CHUNK: segments
# per mid-stream chunk.  RSQ_GROUPS: sqrt/recip batches over segment
# ranges (seg 15 chain is emitted between batch boundaries per order).
CFG = dict(
    splits=(14, 15),
    tail=256,
    full_eng={0: "d", 1: "a", 2: "p", 3: "d", 4: "a", 5: "p",
              6: "d", 7: "a", 8: "p", 9: "a", 10: "d", 11: "a",
              12: "p", 13: "d"},
    fold_eng={14: "a"},
    tail_chunks=((14, 15),),
    full_per_chunk=2,
    full_chunks=(2, 2, 2, 2, 2, 2, 1),
    rsq_groups=((0, 7), (7, 13), (13, 16)),
)

_NC = None
_LAYOUT = None


def _layout(cfg):
    """Column layout: [heads of splits | full segs | seg0.. | tails]."""
    splits = cfg["splits"]
    tail = cfg["tail"]
    head = D - tail
    fulls = [t for t in range(NSEG) if t not in splits]
    cols = {}          # seg -> list of (flat_lo, seg_lo, width)
    pos = 0
    for t in splits:
        cols[t] = [(pos, 0, head)]
        pos += head
    for t in fulls:
        cols[t] = [(pos, 0, D)]
        pos += D
    mid_end = pos
    for t in splits:
        cols[t].append((pos, head, tail))
        pos += tail
    assert pos == FLAT
    # chunks: first chunk = heads + first full seg, then grouped full
    # segs (full_chunks group sizes, or full_per_chunk round-robin),
    # then the tail chunks.
    bounds = [0]
    first = len(splits) * head + D
    bounds.append(first)
    nfull_rest = len(fulls) - 1
    p = first
    groups = cfg.get("full_chunks")
    if groups is None:
        k = cfg["full_per_chunk"]
        groups = []
        i = 0
        while i < nfull_rest:
            step = min(k, nfull_rest - i)
            groups.append(step)
            i += step
    assert sum(groups) == nfull_rest
    for step in groups:
        p += step * D
        bounds.append(p)
    assert p == mid_end
    for grp in cfg["tail_chunks"]:
        p += len(grp) * tail
        bounds.append(p)
    assert p == FLAT
    return cols, bounds


def _build_nc(cfg=None, debug=False) -> bass.Bass:
    cfg = cfg or CFG
    splits = cfg["splits"]
    tail = cfg["tail"]
    head = D - tail
    cols, bounds = _layout(cfg)
    full_eng = cfg["full_eng"]
    fold_eng = cfg["fold_eng"]

    nc = bacc.Bacc(None)
    x_in = nc.declare_dram_parameter("x", [P, FLAT], F16, isOutput=False)
    s_out = nc.declare_dram_parameter("s", [1, P, 1, DCH], F32, isOutput=True)
    if debug:
        ss_dbg = nc.declare_dram_parameter("ss_dbg", [P, NSEG], F32, isOutput=True)
        rn_dbg = nc.declare_dram_parameter("rn_dbg", [P, NSEG], F16, isOutput=True)
        res_dbg = nc.declare_dram_parameter("res_dbg", [P, DCH], F32, isOutput=True)

    with tile.TileContext(nc) as tc:
        with (
            tc.tile_pool(name="xs", bufs=1) as xs_pool,
            tc.tile_pool(name="scr", bufs=1) as scr_pool,
            tc.tile_pool(name="st", bufs=1) as st_pool,
            tc.tile_pool(name="acc", bufs=1, space="PSUM") as psum_pool,
        ):
            # res first: the kv_writeback prep's descriptors bake this
            # tile's SBUF address, so give it the most stable slot
            res = xs_pool.tile([P, 1, 1, DCH], F32, tag="res")
            xt = xs_pool.tile([P, FLAT], F16, tag="x")
            scr_v = scr_pool.tile([P, D], F16, tag="scr_v")
            scr_d = scr_pool.tile([P, D], F16, tag="scr_d")
            scr_a = scr_pool.tile([P, D], F16, tag="scr_a")
            # Pool squares are reduced cross-engine (DVE); double-buffer so
            # the next Pool square doesn't WAW-wait on the DVE read
            scr_p = scr_pool.tile([P, 2, D], F16, tag="scr_p")
            # separate scratches for tail folds: keeps them off the WAW
            # chains of the mid-stream stats on the same engines
            scr_v2 = scr_pool.tile([P, tail], F16, tag="scr_v2")
            scr_d2 = scr_pool.tile([P, tail], F16, tag="scr_d2")
            scr_a2 = scr_pool.tile([P, tail], F16, tag="scr_a2")
            scr_p2 = scr_pool.tile([P, 2, tail], F16, tag="scr_p2")
            ss = st_pool.tile([P, NSEG], F32, tag="ss")
            ssh = st_pool.tile([P, NSEG], F32, tag="ssh")   # head partials
            sst = st_pool.tile([P, NSEG], F32, tag="sst")   # tail partials
            ri = st_pool.tile([P, NSEG], F32, tag="ri")
            rn = st_pool.tile([P, NSEG], F16, tag="rn")
            acc = psum_pool.tile([P, DCH, D], F32, tag="acc")
            zidx = st_pool.tile([P, 1], I32, tag="zidx")
            dum = st_pool.tile([P, 1], F32, tag="dum")
            gate = st_pool.tile([P, 1], F32, tag="gate")
            gscr = st_pool.tile([P, 1], F32, tag="gscr")

            # Dummy sqrt on a const AP: loads the one activation table
            # (sqrt_and_others: Sqrt + Square) under the DMA head.
            nc.scalar.sqrt(out=dum, in_=nc.const_aps.tensor(0.0, (P, 1)))

            if debug:
                sent = st_pool.tile([P, DCH], F32, tag="sent")
                nc.vector.memset(sent, -777.0)
                nc.sync.dma_start(out=s_out[0, :, 0, :], in_=sent)
                nc.vector.memset(res[:, 0, 0, :], -333.0)

            # SWDGE output path: descriptors prepared now (Pool engine,
            # head shadow; only the ctx-idx metadata is read at prep time);
            # the DMA fires via trigger_dma at the end.  The descriptor-
            # baked completion sem must be Tile's DMASW0 lane sem: the
            # epilogue drain auto-waits on it (>= 16 per tick).
            nc.gpsimd.memset(zidx[:, :], 0)
            nc.vector.memset(gate, 0.0)
            prep = nc.gpsimd.kv_writeback(
                out_ap=s_out[:, :, :, :],
                in_ap=res[:, :, :, :],
                ctx_idxs_ap=zidx[:, :],
                prepare_only=True,
                sem=tc.sems.swdge_block()[0],
            )

            for lo, hi in zip(bounds, bounds[1:]):
                nc.sync.dma_start(out=xt[:, lo:hi], in_=x_in[:, lo:hi])

            pool_buf = [0]

            def square_accum(eng, xap, out_col, sv, sd, sa, sp):
                if eng == "a":
                    nc.scalar.activation(
                        out=sa[:, 0 : xap.shape[1]],
                        in_=xap,
                        func=SQUARE,
                        accum_out=out_col,
                    )
                elif eng == "p":
                    # TensorScalarPtr is not a legal Pool opcode, so Pool
                    # only squares; DVE does the cheap 4x-mode reduce
                    w = xap.shape[1]
                    k = pool_buf[0]
                    pool_buf[0] ^= 1
                    nc.gpsimd.tensor_mul(sp[:, k, 0:w], xap, xap)
                    nc.vector.tensor_scalar(
                        out=sd[:, 0:w],
                        in0=sp[:, k, 0:w],
                        scalar1=1.0,
                        scalar2=0.0,
                        op0=MULT,
                        op1=ADD,
                        accum_out=out_col,
                    )
                else:
                    w = xap.shape[1]
                    nc.vector.tensor_mul(sv[:, 0:w], xap, xap)
                    nc.vector.tensor_scalar(
                        out=sd[:, 0:w],
                        in0=sv[:, 0:w],
                        scalar1=1.0,
                        scalar2=0.0,
                        op0=MULT,
                        op1=ADD,
                        accum_out=out_col,
                    )

            def seg_ap(t, part):
                # part: 0 = head piece, 1 = tail piece (splits only)
                lo, _slo, w = cols[t][part]
                return xt[:, lo : lo + w]

            def emit_full(t):
                square_accum(
                    full_eng[t], seg_ap(t, 0), ss[:, t : t + 1],
                    scr_v, scr_d, scr_a, scr_p,
                )

            def emit_head(t):
                # head partial of a split seg (always DVE: 2x/4x, early)
                square_accum(
                    "d", seg_ap(t, 0), ssh[:, t : t + 1],
                    scr_v, scr_d, scr_a, scr_p,
                )

            def emit_fold(t):
                # (tensor_tensor_reduce would fold tail+init in one op but
                # crashes the device runtime, so: square+reduce into sst,
                # then a tiny add merges head+tail partials)
                eng = fold_eng.get(t, "d")
                square_accum(
                    eng, seg_ap(t, 1), sst[:, t : t + 1],
                    scr_v2, scr_d2, scr_a2, scr_p2,
                )
                return False

            def emit_mm(t):
                for c in range(DCH):
                    piece = 0 if (t not in splits or c * P < head) else 1
                    lo, slo, _w = cols[t][piece]
                    off = lo + c * P - (0 if piece == 0 else slo)
                    nc.tensor.matmul(
                        acc[:, c, 0:1],
                        lhsT=xt[:, off : off + P],
                        rhs=rn[:, t : t + 1],
                        start=(t == 0),
                        stop=(t == NSEG - 1),
                    )

            # head partials first (they ride the first chunk)
            for t in splits:
                emit_head(t)

            for lo, hi in cfg["rsq_groups"]:
                need_merge = []
                for t in range(lo, hi):
                    if t in splits:
                        if not emit_fold(t):
                            need_merge.append(t)
                    else:
                        emit_full(t)
                # merge head+tail partials for square_accum-style folds
                for t in need_merge:
                    nc.vector.tensor_tensor(
                        out=ss[:, t : t + 1],
                        in0=ssh[:, t : t + 1],
                        in1=sst[:, t : t + 1],
                        op=ADD,
                    )
                nc.scalar.sqrt(out=ri[:, lo:hi], in_=ss[:, lo:hi])
                with nc.allow_low_precision(reason="fp16 rnorm for PE rhs"):
                    nc.vector.reciprocal(out=rn[:, lo:hi], in_=ri[:, lo:hi])
                for t in range(lo, hi):
                    emit_mm(t)

            # The prep was emitted before res existed, so Tile's deferred
            # source-read edge binds to the pre-write version of res; gate
            # the trigger on the res write explicitly.
            # Gate the trigger on the res write: the prep predates res (its
            # deferred-read edge is useless), and compute ops only carry a
            # single sem update, so a Pool-engine read of res carries the
            # then_inc (ucode handles multiple updates) and the trigger
            # waits on it from the Pool sequencer.
            copy = nc.vector.tensor_copy(res[:, 0, 0, :], acc[:, :, 0])
            # The copy overwrites the res version the early prep deferred-
            # reads, so Tile orders it after the output DMA — a circular
            # WAR edge.  Demote it: the trigger gate below keeps the DMA
            # (which physically reads at trigger time) after the copy, so
            # the version bookkeeping edge is vacuous.
            import bass_rust as _br
            for dep in (prep.ins.name,):
                if copy.ins.has_dependency(dep):
                    copy.ins.remove_dependency(dep)
                    copy.ins.add_dependency(dep, _br.DependencyInfo.NO_SYNC_ONLY)
            # Gate the trigger on the res copy with a WAR edge through a
            # dedicated gate tile: a DVE op ordered after the copy reads
            # the gate, and the trigger declares a write of it.  The edge
            # rides Tile's own lane sems (reset every launch) — a user
            # semaphore's stale cross-execution state let the trigger fire
            # early on hardware.  (Declaring res itself written would make
            # the trigger WAR-wait on the prep's deferred read — circular.)
            nc.vector.tensor_scalar_mul(gate, res[:, 0, 0, 0:1], 1.0)
            nc.gpsimd.trigger_dma(count=None, signals_writable=[gate[:, :]])
            if debug:
                nc.sync.dma_start(out=ss_dbg[:, :], in_=ss)
                nc.sync.dma_start(out=rn_dbg[:, :], in_=rn)
                nc.sync.dma_start(out=res_dbg[:, :], in_=res[:, 0, 0, :])

    nc.finalize()
    return nc


def _shard(x: np.ndarray, cfg=None) -> list[dict]:
    cfg = cfg or CFG
    cols, _ = _layout(cfg)
    xh = np.ascontiguousarray(x, dtype=np.float32).astype(np.float16)
    xh = xh.reshape(NCORES, P, NSEG, D)
    flat = np.empty((NCORES, P, FLAT), dtype=np.float16)
    for t, pieces in cols.items():
        for flo, slo, w in pieces:
            flat[:, :, flo : flo + w] = xh[:, :, t, slo : slo + w]
    return [{"x": flat[c]} for c in range(NCORES)]


def _loss_from_results(results) -> np.ndarray:
    s = np.zeros(D, dtype=np.float64)
    for r in results:
        # s_out[0, p, 0, c] = s[c*128 + p]
        s += r["s"].reshape(P, DCH).T.reshape(D).astype(np.float64)
    loss = (float(s @ s) - float(N)) / (N * TEMPERATURE)
    return np.asarray(loss, dtype=np.float32)


def _run(x: np.ndarray, trace: bool = False):
    global _NC
    if _NC is None:
        _NC = _build_nc()
    out = run_bass_kernel_spmd(
        _NC, _shard(x), core_ids=list(range(NCORES)), trace=trace
    )
    return _loss_from_results(out.results), out


def kernel(x: np.ndarray) -> np.ndarray:
    # Device executions of the same NEFF on the same inputs are
    # deterministic, so two agreeing runs rule out the sporadic
    # first-execution corruption seen on the axon/PJRT path.
    prev = None
    for _ in range(4):
        loss, _out = _run(x)
        if prev is not None and loss == prev:
            return loss
        prev = loss
    return prev
